# revision 5
# baseline (speedup 1.0000x reference)
"""MoE expert-combine kernel for Trainium2 (Bass/Tile), 8-core SPMD.

Problem: out[b,s,:] = sum_k expert_weights[b,s,k] * expert_outputs[expert_indices[b,s,k], b, s, :]
  B,S,H = 4,2048,1024 ; E=8 ; K=2  (hidden_states is unused by the reference)

Sharding: flatten tokens t = b*S+s (8192 total), give each of the 8 cores a
contiguous block of 1024 tokens. Each core receives the expert-output stack
sliced to its tokens ([E, 1024, H] viewed as a row table [E*1024, H]) plus
host-precomputed gather row indices (idx[t,k]*1024 + t_local) and weights.
On-device: indirect-DMA gather of the K=2 selected 4KB rows per token,
weighted combine on the vector engine, store. Memory traffic per core is
8MB gathered + 4MB written (vs 32MB for a dense all-expert read).
"""

import sys
import numpy as np

for _p in ("/opt/trn_rl_repo", "/opt/pypackages"):
    if _p not in sys.path:
        sys.path.append(_p)

from concourse import bacc, bass, mybir
import concourse.tile as tile
from concourse.bass_utils import run_bass_kernel_spmd

B, S, H = 4, 2048, 1024
E, K = 8, 2
N_CORES = 8
T = B * S              # 8192 tokens total
TC = T // N_CORES      # 1024 tokens per core
P = 128                # SBUF partitions
NCHUNK = TC // P       # 8 chunks of 128 tokens per core

_f32 = mybir.dt.float32
_i32 = mybir.dt.int32


def _build():
    # Bacc (not raw Bass): its compile() pass splits multi-sem waits into
    # event-semaphore instructions — walrus codegen allows at most one sync
    # wait per compute instruction.
    nc = bacc.Bacc(None, target_bir_lowering=False)
    table = nc.declare_dram_parameter("table", [E * TC, H], _f32, isOutput=False)
    idx = nc.declare_dram_parameter("idx", [P, NCHUNK * K], _i32, isOutput=False)
    wgt = nc.declare_dram_parameter("wgt", [P, NCHUNK * K], _f32, isOutput=False)
    out = nc.declare_dram_parameter("out", [TC, H], _f32, isOutput=True)

    with tile.TileContext(nc) as tc:
        with (
            tc.tile_pool(name="io", bufs=1) as io_pool,
            tc.tile_pool(name="g", bufs=3) as g_pool,
            tc.tile_pool(name="o", bufs=3) as o_pool,
        ):
            idx_t = io_pool.tile([P, NCHUNK * K], _i32)
            w_t = io_pool.tile([P, NCHUNK * K], _f32)
            nc.sync.dma_start(out=idx_t[:], in_=idx[:])
            nc.sync.dma_start(out=w_t[:], in_=wgt[:])
            for c in range(NCHUNK):
                # gather both selected expert rows for the 128 tokens of
                # chunk c: g[p, k*H:(k+1)*H] = table[idx_t[p, c*K+k], :].
                # NOTE: the HW DGE consumes exactly one index per partition,
                # so each k needs its own indirect DMA with a [P,1] offset.
                g = g_pool.tile([P, K * H], _f32)
                for k in range(K):
                    nc.gpsimd.indirect_dma_start(
                        out=g[:, k * H : (k + 1) * H],
                        out_offset=None,
                        in_=table[:],
                        in_offset=bass.IndirectOffsetOnAxis(
                            ap=idx_t[:, c * K + k : c * K + k + 1], axis=0
                        ),
                    )
                acc = o_pool.tile([P, H], _f32)
                ot = o_pool.tile([P, H], _f32)
                nc.vector.tensor_scalar_mul(
                    acc[:], g[:, 0:H], w_t[:, c * K : c * K + 1]
                )
                nc.vector.scalar_tensor_tensor(
                    out=ot[:],
                    in0=g[:, H : 2 * H],
                    scalar=w_t[:, c * K + 1 : c * K + 2],
                    in1=acc[:],
                    op0=mybir.AluOpType.mult,
                    op1=mybir.AluOpType.add,
                )
                nc.sync.dma_start(out=out[c * P : (c + 1) * P, :], in_=ot[:])
    nc.finalize()  # Bacc: runs compile() passes (multi-wait split, reg alloc)
    return nc


def _prepare_in_maps(expert_indices, expert_weights, expert_outputs):
    eo = np.ascontiguousarray(np.asarray(expert_outputs, dtype=np.float32)).reshape(
        E, T, H
    )
    flat_idx = np.asarray(expert_indices).reshape(T, K).astype(np.int32)
    flat_w = np.asarray(expert_weights, dtype=np.float32).reshape(T, K)
    t_local = np.arange(TC, dtype=np.int32)[:, None]
    in_maps = []
    for i in range(N_CORES):
        t0 = i * TC
        slab = np.ascontiguousarray(eo[:, t0 : t0 + TC, :]).reshape(E * TC, H)
        li = flat_idx[t0 : t0 + TC] * TC + t_local  # [TC, K] row idx into slab
        li = np.ascontiguousarray(
            li.reshape(NCHUNK, P, K).transpose(1, 0, 2).reshape(P, NCHUNK * K)
        )
        w = np.ascontiguousarray(
            flat_w[t0 : t0 + TC]
            .reshape(NCHUNK, P, K)
            .transpose(1, 0, 2)
            .reshape(P, NCHUNK * K)
        )
        in_maps.append({"table": slab, "idx": li, "wgt": w})
    return in_maps


def run(
    hidden_states,
    expert_indices,
    expert_weights,
    expert_outputs,
    trace=False,
):
    in_maps = _prepare_in_maps(expert_indices, expert_weights, expert_outputs)
    nc = _build()
    res = run_bass_kernel_spmd(nc, in_maps, list(range(N_CORES)), trace=trace)
    outs = [np.asarray(res.results[i]["out"]) for i in range(N_CORES)]
    full = np.concatenate(outs, axis=0).reshape(B, S, H).astype(np.float32)
    return full, res


def kernel(hidden_states, expert_indices, expert_weights, expert_outputs):
    full, _ = run(hidden_states, expert_indices, expert_weights, expert_outputs)
    return full


# revision 6
# speedup vs baseline: 1.0028x; 1.0028x over previous
"""MoE expert-combine kernel for Trainium2 (Bass/Tile), 8-core SPMD.

Problem: out[b,s,:] = sum_k expert_weights[b,s,k] * expert_outputs[expert_indices[b,s,k], b, s, :]
  B,S,H = 4,2048,1024 ; E=8 ; K=2  (hidden_states is unused by the reference)

Sharding: flatten tokens t = b*S+s (8192 total), give each of the 8 cores a
contiguous block of 1024 tokens. Each core receives the expert-output stack
sliced to its tokens ([E, 1024, H] viewed as a row table [E*1024, H]) plus
host-precomputed gather row indices (idx[t,k]*1024 + t_local) and weights.
On-device: indirect-DMA gather of the K=2 selected 4KB rows per token,
weighted combine on the vector engine, store. Memory traffic per core is
8MB gathered + 4MB written (vs 32MB for a dense all-expert read).
"""

import sys
import numpy as np

for _p in ("/opt/trn_rl_repo", "/opt/pypackages"):
    if _p not in sys.path:
        sys.path.append(_p)

from concourse import bacc, bass, mybir
import concourse.tile as tile
from concourse.bass_utils import run_bass_kernel_spmd

B, S, H = 4, 2048, 1024
E, K = 8, 2
N_CORES = 8
T = B * S              # 8192 tokens total
TC = T // N_CORES      # 1024 tokens per core
P = 128                # SBUF partitions
NCHUNK = TC // P       # 8 chunks of 128 tokens per core

_f32 = mybir.dt.float32
_i32 = mybir.dt.int32


def _build():
    # Bacc (not raw Bass): its compile() pass splits multi-sem waits into
    # event-semaphore instructions — walrus codegen allows at most one sync
    # wait per compute instruction.
    nc = bacc.Bacc(None, target_bir_lowering=False)
    table = nc.declare_dram_parameter("table", [E * TC, H], _f32, isOutput=False)
    idx = nc.declare_dram_parameter("idx", [P, NCHUNK * K], _i32, isOutput=False)
    wgt = nc.declare_dram_parameter("wgt", [P, NCHUNK * K], _f32, isOutput=False)
    out = nc.declare_dram_parameter("out", [TC, H], _f32, isOutput=True)

    with tile.TileContext(nc) as tc:
        with (
            tc.tile_pool(name="io", bufs=1) as io_pool,
            tc.tile_pool(name="g", bufs=8) as g_pool,
            tc.tile_pool(name="o", bufs=6) as o_pool,
        ):
            idx_t = io_pool.tile([P, NCHUNK * K], _i32)
            w_t = io_pool.tile([P, NCHUNK * K], _f32)
            nc.sync.dma_start(out=idx_t[:], in_=idx[:])
            nc.sync.dma_start(out=w_t[:], in_=wgt[:])
            for c in range(NCHUNK):
                # gather both selected expert rows for the 128 tokens of
                # chunk c: g[p, k*H:(k+1)*H] = table[idx_t[p, c*K+k], :].
                # NOTE: the HW DGE consumes exactly one index per partition,
                # so each k needs its own indirect DMA with a [P,1] offset.
                g = g_pool.tile([P, K * H], _f32)
                for k in range(K):
                    nc.gpsimd.indirect_dma_start(
                        out=g[:, k * H : (k + 1) * H],
                        out_offset=None,
                        in_=table[:],
                        in_offset=bass.IndirectOffsetOnAxis(
                            ap=idx_t[:, c * K + k : c * K + k + 1], axis=0
                        ),
                    )
                acc = o_pool.tile([P, H], _f32)
                ot = o_pool.tile([P, H], _f32)
                nc.vector.tensor_scalar_mul(
                    acc[:], g[:, 0:H], w_t[:, c * K : c * K + 1]
                )
                nc.vector.scalar_tensor_tensor(
                    out=ot[:],
                    in0=g[:, H : 2 * H],
                    scalar=w_t[:, c * K + 1 : c * K + 2],
                    in1=acc[:],
                    op0=mybir.AluOpType.mult,
                    op1=mybir.AluOpType.add,
                )
                nc.sync.dma_start(out=out[c * P : (c + 1) * P, :], in_=ot[:])
    nc.finalize()  # Bacc: runs compile() passes (multi-wait split, reg alloc)
    return nc


def _prepare_in_maps(expert_indices, expert_weights, expert_outputs):
    eo = np.ascontiguousarray(np.asarray(expert_outputs, dtype=np.float32)).reshape(
        E, T, H
    )
    flat_idx = np.asarray(expert_indices).reshape(T, K).astype(np.int32)
    flat_w = np.asarray(expert_weights, dtype=np.float32).reshape(T, K)
    t_local = np.arange(TC, dtype=np.int32)[:, None]
    in_maps = []
    for i in range(N_CORES):
        t0 = i * TC
        slab = np.ascontiguousarray(eo[:, t0 : t0 + TC, :]).reshape(E * TC, H)
        li = flat_idx[t0 : t0 + TC] * TC + t_local  # [TC, K] row idx into slab
        li = np.ascontiguousarray(
            li.reshape(NCHUNK, P, K).transpose(1, 0, 2).reshape(P, NCHUNK * K)
        )
        w = np.ascontiguousarray(
            flat_w[t0 : t0 + TC]
            .reshape(NCHUNK, P, K)
            .transpose(1, 0, 2)
            .reshape(P, NCHUNK * K)
        )
        in_maps.append({"table": slab, "idx": li, "wgt": w})
    return in_maps


def run(
    hidden_states,
    expert_indices,
    expert_weights,
    expert_outputs,
    trace=False,
):
    in_maps = _prepare_in_maps(expert_indices, expert_weights, expert_outputs)
    nc = _build()
    res = run_bass_kernel_spmd(nc, in_maps, list(range(N_CORES)), trace=trace)
    outs = [np.asarray(res.results[i]["out"]) for i in range(N_CORES)]
    full = np.concatenate(outs, axis=0).reshape(B, S, H).astype(np.float32)
    return full, res


def kernel(hidden_states, expert_indices, expert_weights, expert_outputs):
    full, _ = run(hidden_states, expert_indices, expert_weights, expert_outputs)
    return full


# revision 9
# speedup vs baseline: 1.0897x; 1.0867x over previous
"""MoE expert-combine kernel for Trainium2 (raw Bass, hand-scheduled), 8-core SPMD.

Problem: out[b,s,:] = sum_k expert_weights[b,s,k] * expert_outputs[expert_indices[b,s,k], b, s, :]
  B,S,H = 4,2048,1024 ; E=8 ; K=2  (hidden_states is unused by the reference)

Sharding: flatten tokens t = b*S+s (8192 total); each of the 8 cores owns a
contiguous block of 1024 tokens. Each core receives the expert-output stack
sliced to its tokens ([E, 1024, H] viewed as a row table [E*1024, H]) plus
host-precomputed gather row indices (idx[t,k]*1024 + t_local) and weights,
packed into one [128, 32] int32 tensor (16 idx cols + 16 f32-bitcast weight
cols). On-device per 128-token chunk: two indirect-DMA gathers of the
selected 4KB rows (Pool/SWDGE), weighted combine on DVE
(w0*g0 via tensor_scalar, then (w1*g1)+acc via scalar_tensor_tensor), and an
HWDGE store. Hand-placed semaphores; at most one sync-wait per compute
instruction (walrus codegen limit). No Tile framework: avoids its
event-semaphore chains and the multi-microsecond tail drain/barrier/sem-clear.
"""

import sys
import numpy as np

for _p in ("/opt/trn_rl_repo", "/opt/pypackages"):
    if _p not in sys.path:
        sys.path.append(_p)

from concourse import bass, mybir
from concourse.bass_utils import run_bass_kernel_spmd

B, S, H = 4, 2048, 1024
E, K = 8, 2
N_CORES = 8
T = B * S              # 8192 tokens total
TC = T // N_CORES      # 1024 tokens per core
P = 128                # SBUF partitions
NCHUNK = TC // P       # 8 chunks of 128 tokens per core

_f32 = mybir.dt.float32
_i32 = mybir.dt.int32


def _build():
    nc = bass.Bass(target_bir_lowering=False)

    table = nc.declare_dram_parameter("table", [E * TC, H], _f32, isOutput=False)
    idxw = nc.declare_dram_parameter("idxw", [P, 2 * NCHUNK * K], _i32, isOutput=False)
    out = nc.declare_dram_parameter("out", [TC, H], _f32, isOutput=True)

    with (
        nc.Block() as block,
        nc.semaphore("sem_in") as sem_in,
        nc.semaphore("sem_v") as sem_v,
        nc.semaphore("sem_st") as sem_st,
        nc.sbuf_tensor("idxw_t", [P, 2 * NCHUNK * K], _i32) as idxw_t,
        nc.sbuf_tensor("g_t", [P, NCHUNK * K * H], _f32) as g_t,
        nc.sbuf_tensor("ot_t", [P, NCHUNK * H], _f32) as ot_t,
        nc.sbuf_tensor("acc_t", [P, H], _f32) as acc_t,
    ):
        gather_sems = [nc.alloc_semaphore(f"sem_g{i}") for i in range(NCHUNK * K)]

        @block.sync
        def _(sync: bass.BassEngine):
            sync.dma_start(out=idxw_t[:], in_=idxw[:]).then_inc(sem_in, 16)
            for c in range(NCHUNK):
                # ot chunk c is ready after DVE op 2c+2 (1 sem inc per op)
                sync.wait_ge(sem_v, 2 * c + 2)
                sync.dma_start(
                    out=out[c * P : (c + 1) * P, :],
                    in_=ot_t[:, c * H : (c + 1) * H],
                ).then_inc(sem_st, 16)
            sync.wait_ge(sem_st, 16 * NCHUNK)

        @block.gpsimd
        def _(gpsimd: bass.BassEngine):
            gpsimd.wait_ge(sem_in, 16)
            for c in range(NCHUNK):
                for k in range(K):
                    m = c * K + k
                    gpsimd.indirect_dma_start(
                        out=g_t[:, m * H : (m + 1) * H],
                        out_offset=None,
                        in_=table[:],
                        in_offset=bass.IndirectOffsetOnAxis(
                            ap=idxw_t[:, m : m + 1], axis=0
                        ),
                    ).then_inc(gather_sems[m], 16)

        @block.vector
        def _(vector: bass.BassEngine):
            for c in range(NCHUNK):
                m0, m1 = c * K, c * K + 1
                w0 = idxw_t[:, NCHUNK * K + m0 : NCHUNK * K + m0 + 1].bitcast(_f32)
                w1 = idxw_t[:, NCHUNK * K + m1 : NCHUNK * K + m1 + 1].bitcast(_f32)
                # w cols arrive in the same DMA the gathers waited on, so a
                # completed gather implies the weights are resident too.
                vector.tensor_scalar(
                    out=acc_t[:],
                    in0=g_t[:, m0 * H : (m0 + 1) * H],
                    scalar1=w0,
                    scalar2=None,
                    op0=mybir.AluOpType.mult,
                )._wait_ge(gather_sems[m0], 16).then_inc(sem_v, 1)
                vector.scalar_tensor_tensor(
                    out=ot_t[:, c * H : (c + 1) * H],
                    in0=g_t[:, m1 * H : (m1 + 1) * H],
                    scalar=w1,
                    in1=acc_t[:],
                    op0=mybir.AluOpType.mult,
                    op1=mybir.AluOpType.add,
                )._wait_ge(gather_sems[m1], 16).then_inc(sem_v, 1)

    nc.finalize()
    return nc


def _prepare_in_maps(expert_indices, expert_weights, expert_outputs):
    eo = np.ascontiguousarray(np.asarray(expert_outputs, dtype=np.float32)).reshape(
        E, T, H
    )
    flat_idx = np.asarray(expert_indices).reshape(T, K).astype(np.int32)
    flat_w = np.asarray(expert_weights, dtype=np.float32).reshape(T, K)
    t_local = np.arange(TC, dtype=np.int32)[:, None]
    in_maps = []
    for i in range(N_CORES):
        t0 = i * TC
        slab = np.ascontiguousarray(eo[:, t0 : t0 + TC, :]).reshape(E * TC, H)
        li = flat_idx[t0 : t0 + TC] * TC + t_local  # [TC, K] row idx into slab
        li = li.reshape(NCHUNK, P, K).transpose(1, 0, 2).reshape(P, NCHUNK * K)
        w = (
            flat_w[t0 : t0 + TC]
            .reshape(NCHUNK, P, K)
            .transpose(1, 0, 2)
            .reshape(P, NCHUNK * K)
        )
        idxw = np.empty((P, 2 * NCHUNK * K), dtype=np.int32)
        idxw[:, : NCHUNK * K] = li
        idxw[:, NCHUNK * K :] = np.ascontiguousarray(w.astype(np.float32)).view(
            np.int32
        )
        in_maps.append({"table": slab, "idxw": idxw})
    return in_maps


def run(
    hidden_states,
    expert_indices,
    expert_weights,
    expert_outputs,
    trace=False,
):
    in_maps = _prepare_in_maps(expert_indices, expert_weights, expert_outputs)
    nc = _build()
    res = run_bass_kernel_spmd(nc, in_maps, list(range(N_CORES)), trace=trace)
    outs = [np.asarray(res.results[i]["out"]) for i in range(N_CORES)]
    full = np.concatenate(outs, axis=0).reshape(B, S, H).astype(np.float32)
    return full, res


def kernel(hidden_states, expert_indices, expert_weights, expert_outputs):
    full, _ = run(hidden_states, expert_indices, expert_weights, expert_outputs)
    return full
